# revision 1
# baseline (speedup 1.0000x reference)
"""GRU kernel for Trainium2, 8 NeuronCores (SPMD via bass/Tile).

Strategy: tensor-parallel over hidden dim H (128 columns per core), with the
whole recurrence kept in "T-layout" (H on partitions, batch on free dim) so
no per-step transposes are needed:

  phase 0: each core transposes its T/8 slice of x (PE transpose) and the
           slices are AllGathered -> every core holds x^T.
  phase 1: each core computes xprojT[g] = (x @ wxh_g)^T for its own
           128-column slice of H, for all (t, b); stored to DRAM.
           (gate bias is folded into phase 2's activation bias.)
  phase 2: 512 sequential GRU steps. Per step and per batch-stream:
           r/u/c gate matmuls with the gate weight slice stationary
           (lhsT = whh[kt, Sj]) and the gathered h^T streaming; xprojT is
           accumulated into PSUM with an identity-matmul; sigmoid/tanh on
           ScalarE with the gate bias as activation bias; h_new elementwise
           on VectorE.  Cross-core: AllGather of h^T slices and of (r*h)^T
           slices each step.  S phase-shifted batch streams hide the
           collective latency.

The full (unsharded) inputs come in; sharding happens host-side into
per-core in_maps; output slices are reassembled host-side.
"""

import numpy as np

import concourse.bass as bass
import concourse.mybir as mybir
import concourse.tile as tile
from concourse import bacc
from concourse import bass_utils
from concourse.masks import make_identity

F32 = mybir.dt.float32
BF16 = mybir.dt.bfloat16
AF = mybir.ActivationFunctionType

B = 64
D = 512
H = 1024
NC = 8
KT = H // 128          # 8 k-tiles over H
DT = D // 128          # 4 k-tiles over D
RG = [list(range(NC))]


def build_gru(T=512, S=2, mm_bf16=False, timing_reps=1, timing_mode=False):
    """Build the SPMD program. S = number of batch streams (B/S per stream)."""
    TL = T // NC       # timesteps transposed per core in phase 0
    Nb = B // S        # batch per stream
    MMDT = BF16 if mm_bf16 else F32

    nc = bacc.Bacc("TRN2", target_bir_lowering=False, debug=False, num_devices=NC)

    # ---- per-core external inputs (sharded host-side) ----
    x_kind = "Internal" if timing_mode else "ExternalInput"
    x_sl = nc.dram_tensor("x_sl", [B, TL, D], F32, kind=x_kind)
    h0 = nc.dram_tensor("h0", [B, H], F32, kind="ExternalInput")
    whh = nc.dram_tensor("whh", [3, H, 128], F32, kind="ExternalInput")  # r,u,c
    wxh = nc.dram_tensor("wxh", [3, D, 128], F32, kind="ExternalInput")
    bias = nc.dram_tensor("bias", [3, 128], F32, kind="ExternalInput")
    own_h0 = nc.dram_tensor("own_h0", [128, B], F32, kind="ExternalInput")
    T_out = 1 if timing_mode else T
    out = nc.dram_tensor("out", [T_out, 128, B], F32, kind="ExternalOutput")

    # ---- internal DRAM ----
    xT_part = nc.dram_tensor("xT_part", [DT, 128, TL * B], F32, kind="Internal")
    xT_all = nc.dram_tensor("xT_all", [NC, DT, 128, TL * B], F32,
                            kind="Internal", addr_space="Shared")
    xprojT = nc.dram_tensor("xprojT", [3, 128, T, B], F32, kind="Internal")
    agin_h = [nc.dram_tensor(f"agin_h{s}", [128, Nb], MMDT, kind="Internal")
              for s in range(S)]
    agout_h = [nc.dram_tensor(f"agout_h{s}", [H, Nb], MMDT, kind="Internal",
                              addr_space="Shared") for s in range(S)]
    agin_rh = [nc.dram_tensor(f"agin_rh{s}", [128, Nb], MMDT, kind="Internal")
               for s in range(S)]
    agout_rh = [nc.dram_tensor(f"agout_rh{s}", [H, Nb], MMDT, kind="Internal",
                               addr_space="Shared") for s in range(S)]

    with tile.TileContext(nc) as tc:
        with tc.tile_pool(name="const", bufs=1) as cpool, \
             tc.tile_pool(name="w", bufs=1) as wpool, \
             tc.tile_pool(name="st", bufs=2) as stp:

            ident = cpool.tile([128, 128], F32)
            make_identity(nc, ident[:])

            # ============ phases 0+1 (scoped PSUM/SBUF pools) ============
            with tc.tile_pool(name="ph01", bufs=3) as p01, \
                 tc.tile_pool(name="ps01", bufs=2, space="PSUM") as ps01:

                # ---- phase 0: transpose own x slice ----
                for tl in range(TL):
                    xrow = p01.tile([B, D], F32, tag="xrow")
                    nc.sync.dma_start(xrow[:], x_sl.ap()[:, tl, :])
                    for dt in range(DT):
                        ps = ps01.tile([128, B], F32, tag="tp")
                        nc.tensor.transpose(
                            ps[:], xrow[:, dt * 128:(dt + 1) * 128],
                            ident[0:B, 0:B])
                        xc = p01.tile([128, B], F32, tag="xc")
                        nc.scalar.activation(xc[:], ps[:], AF.Copy)
                        nc.sync.dma_start(
                            xT_part.ap()[dt, :, tl * B:(tl + 1) * B], xc[:])

                nc.gpsimd.collective_compute(
                    "AllGather", mybir.AluOpType.bypass, replica_groups=RG,
                    ins=[xT_part.ap()], outs=[xT_all.ap()],
                )

                # ---- phase 1: xprojT for own H-slice ----
                wx_sb = wpool.tile([128, 3 * DT * 128], F32, tag="wx")
                wx_view = wx_sb[:].rearrange("p (g dt m) -> p g dt m", g=3, dt=DT)
                nc.sync.dma_start(
                    wx_view,
                    wxh.ap().rearrange("g (dt p) m -> p g dt m", p=128),
                )
                NCH = min(512, TL * B)       # psum column chunk
                nch_per_c = (TL * B) // NCH  # chunks per gathered block
                for c in range(NC):
                    for ch in range(nch_per_c):
                        cols = slice(ch * NCH, (ch + 1) * NCH)
                        rhs = []
                        for dt in range(DT):
                            rt = p01.tile([128, NCH], F32, tag=f"rhs{dt}")
                            nc.sync.dma_start(rt[:], xT_all.ap()[c, dt, :, cols])
                            rhs.append(rt)
                        for g in range(3):
                            ps = ps01.tile([128, NCH], F32, tag="p1")
                            for dt in range(DT):
                                nc.tensor.matmul(
                                    ps[:], wx_view[:, g, dt, :], rhs[dt][:],
                                    start=(dt == 0), stop=(dt == DT - 1))
                            ot = p01.tile([128, NCH], F32, tag="p1o")
                            nc.scalar.activation(ot[:], ps[:], AF.Copy)
                            nc.sync.dma_start(
                                xprojT.ap().rearrange("g p t b -> g p (t b)")[
                                    g, :, c * TL * B + ch * NCH:
                                    c * TL * B + (ch + 1) * NCH],
                                ot[:])

                # ---- phase-2 constants (uses ps01 for transposes) ----
                wh_sb = wpool.tile([128, 3 * KT * 128], MMDT, tag="wh")
                wh_view = wh_sb[:].rearrange("p (g kt m) -> p g kt m", g=3, kt=KT)
                if mm_bf16:
                    wh_f32 = wpool.tile([128, 3 * KT * 128], F32, tag="whf")
                    nc.sync.dma_start(
                        wh_f32[:].rearrange("p (g kt m) -> p g kt m", g=3, kt=KT),
                        whh.ap().rearrange("g (kt p) m -> p g kt m", p=128),
                    )
                    nc.vector.tensor_copy(wh_sb[:], wh_f32[:])
                else:
                    nc.sync.dma_start(
                        wh_view,
                        whh.ap().rearrange("g (kt p) m -> p g kt m", p=128),
                    )
                bias_sb = cpool.tile([128, 3], F32, tag="bias")
                nc.sync.dma_start(bias_sb[:], bias.ap().rearrange("g p -> p g"))

                # initial h -> T-layout staging [128, KT*B]
                h_nat = cpool.tile([B, H], F32, tag="hnat")
                nc.sync.dma_start(h_nat[:], h0.ap())
                hT0 = cpool.tile([128, KT * B], MMDT, tag="hT0")
                for kt in range(KT):
                    ps = ps01.tile([128, B], F32, tag="tp")
                    nc.tensor.transpose(ps[:], h_nat[:, kt * 128:(kt + 1) * 128],
                                        ident[0:B, 0:B])
                    nc.scalar.activation(hT0[:, kt * B:(kt + 1) * B], ps[:],
                                         AF.Copy)

                # per-stream persistent own-slice h (fp32, exact update)
                h_own = [cpool.tile([128, Nb], F32, tag=f"hown{s}",
                                    name=f"hown{s}") for s in range(S)]
                own_h0_sb = cpool.tile([128, B], F32, tag="ownh0")
                nc.sync.dma_start(own_h0_sb[:], own_h0.ap())
                for s in range(S):
                    nc.vector.tensor_copy(h_own[s][:],
                                          own_h0_sb[:, s * Nb:(s + 1) * Nb])

            # =============== phase 2: the recurrence ===============
            with tc.tile_pool(name="psA", bufs=2, space="PSUM") as psA, \
                 tc.tile_pool(name="psB", bufs=2, space="PSUM") as psB, \
                 tc.tile_pool(name="psC", bufs=2, space="PSUM") as psC:

                sig = AF.Sigmoid
                tnh = AF.Tanh

                for rep in range(timing_reps):
                    for t in range(T):
                        for s in range(S):
                            bs = slice(s * Nb, (s + 1) * Nb)
                            first = (t == 0 and rep == 0)

                            # ---- gathered hT for this step ----
                            if first:
                                def hrhs(kt, s=s):
                                    return hT0[:, kt * B + s * Nb:
                                               kt * B + (s + 1) * Nb]
                            else:
                                hT_g = stp.tile([128, KT * Nb], MMDT, tag=f"hg{s}",
                                                name=f"hg{s}")
                                nc.sync.dma_start(
                                    hT_g[:].rearrange("p (c b) -> p c b", c=KT),
                                    agout_h[s].ap().rearrange(
                                        "(c p) b -> p c b", p=128),
                                )
                                def hrhs(kt, hT_g=hT_g):
                                    return hT_g[:, kt * Nb:(kt + 1) * Nb]

                            # ---- xprojT tiles for this step ----
                            xp_t = stp.tile([128, 3 * Nb], F32, tag=f"xp{s}",
                                            name=f"xp{s}")
                            nc.sync.dma_start(
                                xp_t[:].rearrange("p (g b) -> p g b", g=3),
                                xprojT.ap()[:, :, t, bs].rearrange(
                                    "g p b -> p g b"),
                            )

                            # ---- r gate (critical path first) ----
                            ps_r = psA.tile([128, Nb], F32, tag="ps_r")
                            for kt in range(KT):
                                nc.tensor.matmul(ps_r[:], wh_view[:, 0, kt, :],
                                                 hrhs(kt), start=(kt == 0),
                                                 stop=False)
                            nc.tensor.matmul(ps_r[:], ident[:], xp_t[:, 0:Nb],
                                             start=False, stop=True)
                            r_sb = stp.tile([128, Nb], F32, tag=f"r{s}",
                                            name=f"r{s}")
                            nc.scalar.activation(r_sb[:], ps_r[:], sig,
                                                 bias=bias_sb[:, 0:1])
                            rh_sb = stp.tile([128, Nb], MMDT, tag=f"rh{s}",
                                             name=f"rh{s}")
                            nc.vector.tensor_mul(rh_sb[:], r_sb[:], h_own[s][:])
                            nc.sync.dma_start(agin_rh[s].ap(), rh_sb[:])
                            nc.gpsimd.collective_compute(
                                "AllGather", mybir.AluOpType.bypass,
                                replica_groups=RG,
                                ins=[agin_rh[s].ap()], outs=[agout_rh[s].ap()],
                            )

                            # ---- u gate (off critical path) ----
                            ps_u = psB.tile([128, Nb], F32, tag="ps_u")
                            for kt in range(KT):
                                nc.tensor.matmul(ps_u[:], wh_view[:, 1, kt, :],
                                                 hrhs(kt), start=(kt == 0),
                                                 stop=False)
                            nc.tensor.matmul(ps_u[:], ident[:],
                                             xp_t[:, Nb:2 * Nb],
                                             start=False, stop=True)
                            u_sb = stp.tile([128, Nb], F32, tag=f"u{s}",
                                            name=f"u{s}")
                            nc.scalar.activation(u_sb[:], ps_u[:], sig,
                                                 bias=bias_sb[:, 1:2])

                            # ---- c gate ----
                            rh_g = stp.tile([128, KT * Nb], MMDT, tag=f"rhg{s}",
                                            name=f"rhg{s}")
                            nc.sync.dma_start(
                                rh_g[:].rearrange("p (c b) -> p c b", c=KT),
                                agout_rh[s].ap().rearrange(
                                    "(c p) b -> p c b", p=128),
                            )
                            ps_c = psC.tile([128, Nb], F32, tag="ps_c")
                            for kt in range(KT):
                                nc.tensor.matmul(ps_c[:], wh_view[:, 2, kt, :],
                                                 rh_g[:, kt * Nb:(kt + 1) * Nb],
                                                 start=(kt == 0), stop=False)
                            nc.tensor.matmul(ps_c[:], ident[:],
                                             xp_t[:, 2 * Nb:3 * Nb],
                                             start=False, stop=True)
                            c_sb = stp.tile([128, Nb], F32, tag=f"c{s}",
                                            name=f"c{s}")
                            nc.scalar.activation(c_sb[:], ps_c[:], tnh,
                                                 bias=bias_sb[:, 2:3])

                            # ---- h_new = h + u*(c-h) ----
                            t1 = stp.tile([128, Nb], F32, tag=f"t1{s}",
                                          name=f"t1{s}")
                            nc.vector.tensor_sub(t1[:], c_sb[:], h_own[s][:])
                            t2 = stp.tile([128, Nb], F32, tag=f"t2{s}",
                                          name=f"t2{s}")
                            nc.vector.tensor_mul(t2[:], u_sb[:], t1[:])
                            nc.vector.tensor_add(h_own[s][:], h_own[s][:], t2[:])

                            # write output slice + feed next AG
                            if mm_bf16:
                                hn_mm = stp.tile([128, Nb], MMDT, tag=f"hnc{s}",
                                                 name=f"hnc{s}")
                                nc.vector.tensor_copy(hn_mm[:], h_own[s][:])
                            else:
                                hn_mm = h_own[s]
                            nc.sync.dma_start(out.ap()[t if not timing_mode else 0, :, bs],
                                              h_own[s][:])
                            if not (t == T - 1 and rep == timing_reps - 1):
                                nc.sync.dma_start(agin_h[s].ap(), hn_mm[:])
                                nc.gpsimd.collective_compute(
                                    "AllGather", mybir.AluOpType.bypass,
                                    replica_groups=RG,
                                    ins=[agin_h[s].ap()], outs=[agout_h[s].ap()],
                                )

    nc.compile()
    return nc


_CACHE = {}

S_DEFAULT = 2
BF16_DEFAULT = False


def _get_nc(T=512, S=2, mm_bf16=False, timing_reps=1, timing_mode=False):
    key = (T, S, mm_bf16, timing_reps, timing_mode)
    if key not in _CACHE:
        _CACHE[key] = build_gru(T, S, mm_bf16, timing_reps, timing_mode)
    return _CACHE[key]


def make_in_maps(x, h, r_whh, r_wxh, r_b, u_whh, u_wxh, u_b, c_whh, c_wxh, c_b,
                 T=512):
    TL = T // NC
    whh_full = np.stack([r_whh, u_whh, c_whh])    # [3, H, H]
    wxh_full = np.stack([r_wxh, u_wxh, c_wxh])    # [3, D, H]
    b_full = np.stack([r_b, u_b, c_b])            # [3, H]
    in_maps = []
    for j in range(NC):
        sl = slice(j * 128, (j + 1) * 128)
        in_maps.append({
            "x_sl": np.ascontiguousarray(x[:, j * TL:(j + 1) * TL, :]),
            "h0": np.ascontiguousarray(h),
            "whh": np.ascontiguousarray(whh_full[:, :, sl]),
            "wxh": np.ascontiguousarray(wxh_full[:, :, sl]),
            "bias": np.ascontiguousarray(b_full[:, sl]),
            "own_h0": np.ascontiguousarray(h.T[sl, :]),
        })
    return in_maps


def assemble(results, T=512):
    # results[j]["out"]: [T, 128, B] -> full [B, T, H]
    parts = [r["out"].transpose(2, 0, 1) for r in results]   # [B, T, 128]
    return np.concatenate(parts, axis=2)


def kernel(x, h, r_whh, r_wxh, r_b, u_whh, u_wxh, u_b, c_whh, c_wxh, c_b):
    x = np.asarray(x, dtype=np.float32)
    h = np.asarray(h, dtype=np.float32)
    args = [np.asarray(a, dtype=np.float32) for a in
            (r_whh, r_wxh, r_b, u_whh, u_wxh, u_b, c_whh, c_wxh, c_b)]
    T = x.shape[1]
    nc = _get_nc(T=T, S=S_DEFAULT, mm_bf16=BF16_DEFAULT)
    in_maps = make_in_maps(x, h, *args, T=T)
    res = bass_utils.run_bass_kernel_spmd(nc, in_maps, core_ids=list(range(NC)))
    return assemble(res.results, T=T)



# revision 2
# speedup vs baseline: 46.9379x; 46.9379x over previous
"""GRU kernel for Trainium2, 8 NeuronCores — batch-data-parallel.

Final design: batch B=64 is sharded 8 ways (8 rows/core), weights are
replicated, and there are ZERO collectives — under this runtime each
collective_compute costs ~400us (host round-trip), so the tensor-parallel
"AllGather h every step" design in the sharding hint is ~100x off the pace.
fp16 kernel I/O halves host<->device shipping.

Sharding: batch B=64 -> 8 per core; weights replicated. ZERO collectives —
under the axon runtime each collective_compute costs ~400us (host round
trip), so any per-step collective design is ~100x off the pace. Each core
runs the full H=1024 recurrence for its 8 batch rows.

Layout: h kept natural [8, H] (fp32). Gate matmuls are h-stationary:
lhsT = h^T k-tile [128, 8] (fp16), rhs = whh k-slice [128, 512] (fp16),
out PSUM [8, 512] fp32 — 48 matmuls + 3 gates of elementwise per step.
x-projections (x @ wxh + b, all t) are precomputed in phase 1 at full PE
utilization and streamed from DRAM (fp16) during the recurrence.

The recurrence runs in a tc.For_i hardware loop (unroll U) with
loop-var-indexed DMA for xproj[t] reads and out[t] writes, so the whole
program is ~2k instructions regardless of T.
"""

import numpy as np

import concourse.bass as bass
import concourse.mybir as mybir
import concourse.tile as tile
from concourse import bacc
from concourse import bass_utils
from concourse.masks import make_identity

F32 = mybir.dt.float32
F16 = mybir.dt.float16
AF = mybir.ActivationFunctionType
ALU = mybir.AluOpType

B = 64
D = 512
H = 1024
NC = 8
BC = B // NC           # batch per core = 8
KT = H // 128          # 8 k-tiles over H
DTL = D // 128         # 4 k-tiles over D
HH = 512               # H half (one PSUM bank at fp32)


def build_gru(T=512, U=4, timing_reps=1, hw_loop=True, hints=False, stag=False, R=1):
    TB = T * BC            # (t, b) rows per core
    nc = bacc.Bacc("TRN2", target_bir_lowering=False, debug=False,
                   num_devices=NC)

    # ---- per-core external inputs ----
    x_b = nc.dram_tensor("x_b", [BC, T, D], F16, kind="ExternalInput")
    h0_b = nc.dram_tensor("h0_b", [BC, H], F32, kind="ExternalInput")
    whh = nc.dram_tensor("whh", [3, H, H], F16, kind="ExternalInput")
    wxh = nc.dram_tensor("wxh", [3, D, H], F16, kind="ExternalInput")
    bias = nc.dram_tensor("bias", [3, H], F32, kind="ExternalInput")
    out = nc.dram_tensor("out", [T, BC, H], F16, kind="ExternalOutput")
    # xproj reads at step t: [3, BC, H] fp16 slice (b-major layout)
    xproj = nc.dram_tensor("xproj", [3, BC, T, H], F16, kind="Internal")

    with tile.TileContext(nc) as tc:
        with tc.tile_pool(name="const", bufs=1) as cpool, \
             tc.tile_pool(name="w", bufs=1) as wpool, \
             tc.tile_pool(name="st", bufs=2) as stp:

            ident = cpool.tile([128, 128], F32)
            make_identity(nc, ident[:])
            ident16 = cpool.tile([128, 128], F16, tag="ident16")
            nc.vector.tensor_copy(ident16[:], ident[:])

            wh_sb = wpool.tile([128, 3 * KT * H], F16, tag="wh")
            wh_view = wh_sb[:].rearrange("p (g kt h) -> p g kt h", g=3, kt=KT)
            wx_sb = wpool.tile([128, 3 * DTL * H], F16, tag="wx")
            wx_view = wx_sb[:].rearrange("p (g dt h) -> p g dt h", g=3, dt=DTL)

            # bias as matmul rhs: row 0 = bias, rows 1..127 = 0
            bias_sb = cpool.tile([128, 3 * H], F16, tag="bias_sb")
            bias_v = bias_sb[:].rearrange("p (g h) -> p g h", g=3)
            nc.vector.memset(bias_sb[:], 0.0)
            btmp = cpool.tile([1, 3 * H], F32, tag="btmp")
            nc.sync.dma_start(btmp[:], bias.ap().rearrange("g h -> (g h)")
                              .rearrange("(o gh) -> o gh", o=1))
            nc.vector.tensor_copy(bias_sb[0:1, :], btmp[:])
            ones_row = cpool.tile([128, 128], F16, tag="ones_row")
            nc.vector.memset(ones_row[:], 0.0)
            nc.vector.memset(ones_row[0:1, :], 1.0)

            # ---- phase 0: x -> x^T (fp16, SBUF-resident) ----
            xT = wpool.tile([128, DTL * TB], F16, tag="xT")
            xT_v = xT[:].rearrange("p (dt tb) -> p dt tb", dt=DTL)
            NCHK = TB // 128
            with tc.tile_pool(name="ph0", bufs=2) as p0, \
                 tc.tile_pool(name="ps0", bufs=2, space="PSUM") as ps0:
                # weights are shipped fp16: straight DMA loads
                nc.sync.dma_start(
                    wh_view,
                    whh.ap().rearrange("g (kt p) h -> p g kt h", p=128))
                nc.sync.dma_start(
                    wx_view,
                    wxh.ap().rearrange("g (dt p) h -> p g dt h", p=128))
                for ch in range(NCHK):
                    xrow = p0.tile([128, D], F16, tag="xrow")
                    nc.sync.dma_start(
                        xrow[:],
                        x_b.ap().rearrange("b t d -> (b t) d")[
                            ch * 128:(ch + 1) * 128, :])
                    for dt in range(DTL):
                        ps = ps0.tile([128, 128], F16, tag="tp")
                        nc.tensor.transpose(
                            ps[:], xrow[:, dt * 128:(dt + 1) * 128],
                            ident16[:])
                        nc.scalar.activation(
                            xT_v[:, dt, ch * 128:(ch + 1) * 128], ps[:],
                            AF.Copy)

                # ---- phase 1: xproj = x @ wxh + b (fp16 matmuls) ----
                for ch in range(NCHK):
                    for g in range(3):
                        for hh in range(2):
                            hsl = slice(hh * HH, (hh + 1) * HH)
                            ps = ps0.tile([128, HH], F32, tag="p1")
                            for dt in range(DTL):
                                nc.tensor.matmul(
                                    ps[:],
                                    xT_v[:, dt, ch * 128:(ch + 1) * 128],
                                    wx_view[:, g, dt, hsl],
                                    start=(dt == 0), stop=False)
                            nc.tensor.matmul(ps[:], ones_row[:],
                                             bias_v[:, g, hsl],
                                             start=False, stop=True)
                            ot = p0.tile([128, HH], F16, tag="p1o")
                            nc.scalar.activation(ot[:], ps[:], AF.Copy)
                            r0 = ch * 128
                            nc.sync.dma_start(
                                xproj.ap()[g].rearrange(
                                    "b t h -> (b t) h")[r0:r0 + 128, hsl],
                                ot[:])

                # ---- initial h -> h_nat + hT ----
                h_nat = cpool.tile([BC, H], F32, tag="h_nat")
                nc.sync.dma_start(h_nat[:], h0_b.ap())
                hT = cpool.tile([128, KT * BC], F16, tag="hT")
                for kt in range(KT):
                    ps = ps0.tile([128, BC], F32, tag="tph")
                    nc.tensor.transpose(
                        ps[:], h_nat[:, kt * 128:(kt + 1) * 128],
                        ident[0:BC, 0:BC])
                    nc.scalar.activation(hT[:, kt * BC:(kt + 1) * BC], ps[:],
                                         AF.Copy)

            # =============== recurrence (hardware loop) ===============
            with tc.tile_pool(name="psg", bufs=1, space="PSUM") as psg, \
                 tc.tile_pool(name="pst", bufs=2, space="PSUM") as pst:

                def step(t_idx, rh_T, last=False):
                    # t_idx: ScalarValue or int; consumes hT, updates
                    # h_nat/hT in place
                    xp = stp.tile([BC, 3 * H], F16, tag="xp", name="xp")
                    nc.sync.dma_start(
                        xp[:].rearrange("b (g h) -> b g h", g=3),
                        xproj.ap()[:, :, t_idx, :].rearrange(
                            "g b h -> b g h"))
                    xp_v = xp[:].rearrange("b (g h) -> b g h", g=3)

                    ps_t = {}
                    for g in range(3):
                        for hh in range(2):
                            ps_t[(g, hh)] = psg.tile([BC, HH], F32,
                                                     tag=f"ps{g}{hh}",
                                                     name=f"ps{g}{hh}")
                    # r and u gate matmuls (use current hT)
                    for g in range(2):
                        for hh in range(2):
                            hsl = slice(hh * HH, (hh + 1) * HH)
                            for kt in range(KT):
                                nc.tensor.matmul(
                                    ps_t[(g, hh)][:],
                                    hT[:, kt * BC:(kt + 1) * BC],
                                    wh_view[:, g, kt, hsl],
                                    start=(kt == 0), stop=(kt == KT - 1))
                    # r gate elementwise
                    pre_r = stp.tile([BC, H], F32, tag="pre", name="pre_r")
                    for hh in range(2):
                        hsl = slice(hh * HH, (hh + 1) * HH)
                        nc.vector.scalar_tensor_tensor(
                            pre_r[:, hsl], ps_t[(0, hh)][:], 1.0,
                            xp_v[:, 0, hsl], ALU.mult, ALU.add)
                    r_g = stp.tile([BC, H], F32, tag="r_g", name="r_g")
                    nc.scalar.activation(r_g[:], pre_r[:], AF.Sigmoid)
                    rh = stp.tile([BC, H], F32, tag="rh", name="rh")
                    nc.vector.tensor_mul(rh[:], r_g[:], h_nat[:])
                    # rh^T for the c gate
                    for kt in range(KT):
                        ps = pst.tile([128, BC], F32, tag="tp", name="tp")
                        nc.tensor.transpose(
                            ps[:], rh[:, kt * 128:(kt + 1) * 128],
                            ident[0:BC, 0:BC])
                        nc.scalar.activation(
                            rh_T[:, kt * BC:(kt + 1) * BC], ps[:], AF.Copy)
                    # c gate matmuls
                    for hh in range(2):
                        hsl = slice(hh * HH, (hh + 1) * HH)
                        for kt in range(KT):
                            nc.tensor.matmul(
                                ps_t[(2, hh)][:],
                                rh_T[:, kt * BC:(kt + 1) * BC],
                                wh_view[:, 2, kt, hsl],
                                start=(kt == 0), stop=(kt == KT - 1))
                    # u gate elementwise
                    pre_u = stp.tile([BC, H], F32, tag="pre", name="pre_u")
                    for hh in range(2):
                        hsl = slice(hh * HH, (hh + 1) * HH)
                        nc.vector.scalar_tensor_tensor(
                            pre_u[:, hsl], ps_t[(1, hh)][:], 1.0,
                            xp_v[:, 1, hsl], ALU.mult, ALU.add)
                    u_g = stp.tile([BC, H], F32, tag="u_g", name="u_g")
                    nc.scalar.activation(u_g[:], pre_u[:], AF.Sigmoid)
                    # c gate elementwise
                    pre_c = stp.tile([BC, H], F32, tag="pre", name="pre_c")
                    for hh in range(2):
                        hsl = slice(hh * HH, (hh + 1) * HH)
                        nc.vector.scalar_tensor_tensor(
                            pre_c[:, hsl], ps_t[(2, hh)][:], 1.0,
                            xp_v[:, 2, hsl], ALU.mult, ALU.add)
                    c_g = stp.tile([BC, H], F32, tag="c_g", name="c_g")
                    nc.scalar.activation(c_g[:], pre_c[:], AF.Tanh)
                    # h_new = h + u*(c - h)
                    t1 = stp.tile([BC, H], F32, tag="t1", name="t1")
                    nc.vector.tensor_sub(t1[:], c_g[:], h_nat[:])
                    t2 = stp.tile([BC, H], F32, tag="t2", name="t2")
                    nc.vector.tensor_mul(t2[:], u_g[:], t1[:])
                    nc.vector.tensor_add(h_nat[:], h_nat[:], t2[:])
                    hn16 = stp.tile([BC, H], F16, tag="hn16", name="hn16")
                    nc.vector.tensor_copy(hn16[:], h_nat[:])
                    nc.sync.dma_start(out.ap()[t_idx], hn16[:])
                    if not last:
                        for kt in range(KT):
                            ps = pst.tile([128, BC], F32, tag="tp", name="tp")
                            nc.tensor.transpose(
                                ps[:], h_nat[:, kt * 128:(kt + 1) * 128],
                                ident[0:BC, 0:BC])
                            nc.scalar.activation(
                                hT[:, kt * BC:(kt + 1) * BC], ps[:], AF.Copy)

                rh_T = cpool.tile([128, KT * BC], F16, tag="rh_T")
                for rep in range(timing_reps):
                    if hw_loop:
                        import concourse.mybir as _mb
                        kw = {}
                        if hints:
                            kw["hint_engines"] = tuple(_mb.ALL_ENGINES)
                        if stag:
                            kw["staggered_reset"] = True
                        if R == 1:
                            with tc.For_i(0, T, U, **kw) as i:
                                for u in range(U):
                                    step(i + u, rh_T)
                        else:
                            with tc.For_i(0, R, 1) as _r:
                                with tc.For_i(0, T, U, **kw) as i:
                                    for u in range(U):
                                        step(i + u, rh_T)
                    else:
                        for t in range(T):
                            step(t, rh_T)

    nc.compile()
    return nc


_CACHE = {}


def _get_nc(T=512, U=4, timing_reps=1, hw_loop=True, hints=False,
            stag=False, R=1):
    key = (T, U, timing_reps, hw_loop, hints, stag, R)
    if key not in _CACHE:
        _CACHE[key] = build_gru(T, U, timing_reps, hw_loop, hints, stag, R)
    return _CACHE[key]


def make_in_maps(x, h, r_whh, r_wxh, r_b, u_whh, u_wxh, u_b, c_whh, c_wxh, c_b,
                 T=512):
    whh_full = np.ascontiguousarray(np.stack([r_whh, u_whh, c_whh]))
    wxh_full = np.ascontiguousarray(np.stack([r_wxh, u_wxh, c_wxh]))
    b_full = np.ascontiguousarray(np.stack([r_b, u_b, c_b]))
    in_maps = []
    for j in range(NC):
        bsl = slice(j * BC, (j + 1) * BC)
        in_maps.append({
            "x_b": np.ascontiguousarray(x[bsl]).astype(np.float16),
            "h0_b": np.ascontiguousarray(h[bsl]),
            "whh": whh_full.astype(np.float16),
            "wxh": wxh_full.astype(np.float16),
            "bias": b_full,
        })
    return in_maps


def assemble(results, T=512):
    # results[j]["out"]: [T, BC, H] -> full [B, T, H]
    parts = [r["out"].astype(np.float32).transpose(1, 0, 2)
             for r in results]   # [BC, T, H]
    return np.concatenate(parts, axis=0)


def kernel(x, h, r_whh, r_wxh, r_b, u_whh, u_wxh, u_b, c_whh, c_wxh, c_b):
    x = np.asarray(x, dtype=np.float32)
    h = np.asarray(h, dtype=np.float32)
    args = [np.asarray(a, dtype=np.float32) for a in
            (r_whh, r_wxh, r_b, u_whh, u_wxh, u_b, c_whh, c_wxh, c_b)]
    T = x.shape[1]
    nc = _get_nc(T=T)
    in_maps = make_in_maps(x, h, *args, T=T)
    res = bass_utils.run_bass_kernel_spmd(nc, in_maps, core_ids=list(range(NC)))
    return assemble(res.results, T=T)
